# revision 7
# baseline (speedup 1.0000x reference)
"""Trainium2 Bass kernel for masked dot-product attention variant:

    out[b,p,l,m] = (sum_d Q[b,p,l,d] K[b,p,m,d]) / sqrt(D) * mask[b,p] * V[b,p,l,m]

Sharding: data-parallel over batch dim B=16 -> 2 batches per core on 8 cores.
Per core: 128 independent (b,p) pairs, each a 256x128 @ 128x256 fp32 gemm plus
an elementwise multiply with V and a per-pair scalar mask.

Layout trick: Q/K/V/out are DMA'd flat (contiguous 1-2KB runs per partition),
which puts row l = 2*partition + r (r in {0,1}) on each partition. The PE
transposes Q/K chunks through the identity, giving [d, l] tiles whose column
order is the same interleaving. The matmul moving-operand AP un-interleaves
K^T columns so scores come out in natural m order, matching V's layout.

Per (b,p) pair:
  PE : 4x 128x128 fp32 transposes (Q,K chunks -> one PSUM bank) + 2 fp32
       matmuls scores = Qchunk^T.T @ K^T (N=256)
  ACT: 1x [128,512] PSUM->SBUF copy of [Q^T|K^T]
  DVE: 2x fused scalar_tensor_tensor: out = (scores * mask/sqrt(D)) * V
  DMA: 1-2MB group transfers (8 pairs per group), double-buffered

The kernel is DMA-bound: ~100MB HBM traffic per core at ~360GB/s.
"""

import numpy as np

B, P, L, D = 16, 64, 256, 128
NCORES = 8
BPC = B // NCORES          # batches per core = 2
PAIRS = BPC * P            # (b,p) pairs per core = 128
GP = 8                     # pairs per DMA group

ISQRT_D = 1.0 / np.sqrt(D)


def build_bass(pairs=PAIRS):
    import concourse.bacc as bacc
    import concourse.mybir as mybir
    import concourse.tile as tile
    from concourse.bass import ds, ts

    f32 = mybir.dt.float32
    groups = pairs // GP
    nc = bacc.Bacc("TRN2")

    q = nc.dram_tensor("q", [pairs, L * D], f32, kind="ExternalInput")
    k = nc.dram_tensor("k", [pairs, L * D], f32, kind="ExternalInput")
    v = nc.dram_tensor("v", [pairs, L * L], f32, kind="ExternalInput")
    # maskbc[part, pair] = mask[pair] / sqrt(D), same value on all partitions
    maskbc = nc.dram_tensor("maskbc", [128, pairs], f32, kind="ExternalInput")
    ident = nc.dram_tensor("ident", [128, 128], f32, kind="ExternalInput")
    out = nc.dram_tensor("out", [pairs, L * L], f32, kind="ExternalOutput")

    mult = mybir.AluOpType.mult

    with tile.TileContext(nc) as tc:
        with (
            tc.tile_pool(name="const", bufs=1) as cp,
            tc.tile_pool(name="io", bufs=2) as io,
            tc.tile_pool(name="pst", bufs=3, space="PSUM") as pst,
            tc.tile_pool(name="pss", bufs=4, space="PSUM") as pss,
        ):
            ident_sb = cp.tile([128, 128], f32, tag="ident")
            mask_sb = cp.tile([128, pairs], f32, tag="mask")
            nc.sync.dma_start(out=ident_sb[:], in_=ident[:, :])
            nc.sync.dma_start(out=mask_sb[:], in_=maskbc[:, :])

            for g in range(groups):
                sl = slice(g * GP, (g + 1) * GP)
                qn = io.tile([128, GP, 256], f32, tag="qn")
                kn = io.tile([128, GP, 256], f32, tag="kn")
                vn = io.tile([128, GP, 512], f32, tag="vn")
                qkT = io.tile([128, GP, 512], f32, tag="qkT")
                osb = io.tile([128, GP, 512], f32, tag="osb")

                nc.sync.dma_start(
                    out=qn[:], in_=q[sl, :].rearrange("j (p x) -> p j x", p=128)
                )
                nc.sync.dma_start(
                    out=kn[:], in_=k[sl, :].rearrange("j (p x) -> p j x", p=128)
                )
                nc.sync.dma_start(
                    out=vn[:], in_=v[sl, :].rearrange("j (p x) -> p j x", p=128)
                )

                for j in range(GP):
                    pair = g * GP + j
                    # [Q^T (cols 0:256) | K^T (cols 256:512)], one PSUM bank
                    tr = pst.tile([128, 512], f32, tag="tr")
                    for r in range(2):
                        nc.tensor.transpose(
                            tr[:, ts(r, 128)], qn[:, j, ts(r, 128)], ident_sb[:]
                        )
                    for r in range(2):
                        nc.tensor.transpose(
                            tr[:, ds(256 + r * 128, 128)],
                            kn[:, j, ts(r, 128)],
                            ident_sb[:],
                        )
                    nc.scalar.copy(out=qkT[:, j, :], in_=tr[:])

                    # K^T columns are m = 2*p2 + r2; stream them in natural
                    # m order: p2 outer (stride 1), r2 inner (stride 128)
                    kT_mov = qkT[:, j, ds(256, 256)].rearrange(
                        "d (r2 p2) -> d p2 r2", r2=2
                    )
                    sc = pss.tile([128, 512], f32, tag="sc")
                    for r in range(2):
                        nc.tensor.matmul(
                            sc[:, ds(r * 256, 256)],
                            lhsT=qkT[:, j, ts(r, 128)],
                            rhs=kT_mov,
                            start=True,
                            stop=True,
                        )
                        nc.vector.scalar_tensor_tensor(
                            out=osb[:, j, ds(r * 256, 256)],
                            in0=sc[:, ds(r * 256, 256)],
                            scalar=mask_sb[:, ds(pair, 1)],
                            in1=vn[:, j, ds(r * 256, 256)],
                            op0=mult,
                            op1=mult,
                        )

                nc.sync.dma_start(
                    out=out[sl, :].rearrange("j (p x) -> p j x", p=128), in_=osb[:]
                )
    nc.finalize()
    return nc


def make_in_maps(queries, keys, values, mask, ncores=NCORES):
    in_maps = []
    ident = np.eye(128, dtype=np.float32)
    for c in range(ncores):
        bs = slice(c * BPC, (c + 1) * BPC)
        mrow = (mask[bs].reshape(PAIRS) * ISQRT_D).astype(np.float32)
        in_maps.append(
            {
                "q": np.ascontiguousarray(
                    queries[bs].reshape(PAIRS, L * D).astype(np.float32)
                ),
                "k": np.ascontiguousarray(
                    keys[bs].reshape(PAIRS, L * D).astype(np.float32)
                ),
                "v": np.ascontiguousarray(
                    values[bs].reshape(PAIRS, L * L).astype(np.float32)
                ),
                "maskbc": np.ascontiguousarray(
                    np.broadcast_to(mrow[None, :], (128, PAIRS))
                ),
                "ident": ident,
            }
        )
    return in_maps


def run(queries, keys, values, mask, trace=False):
    """Build, compile and run on 8 cores; returns (full_output, BassKernelResults)."""
    from concourse.bass_utils import run_bass_kernel_spmd

    nc = build_bass()
    in_maps = make_in_maps(queries, keys, values, mask)
    res = run_bass_kernel_spmd(
        nc, in_maps, core_ids=list(range(NCORES)), trace=trace
    )
    outs = [r["out"].reshape(BPC, P, L, L) for r in res.results]
    return np.concatenate(outs, axis=0), res


def kernel(queries, keys, values, mask):
    out, _ = run(queries, keys, values, mask, trace=False)
    return out


# revision 8
# speedup vs baseline: 1.0369x; 1.0369x over previous
"""Trainium2 Bass kernel for masked dot-product attention variant:

    out[b,p,l,m] = (sum_d Q[b,p,l,d] K[b,p,m,d]) / sqrt(D) * mask[b,p] * V[b,p,l,m]

Sharding: data-parallel over batch dim B=16 -> 2 batches per core on 8 cores.
Per core: 128 independent (b,p) pairs, each a 256x128 @ 128x256 fp32 gemm plus
an elementwise multiply with V and a per-pair scalar mask.

Host marshalling pre-transposes Q,K to [pair, d, l] layout so the PE matmul
(which contracts along the partition dim) can consume both operands directly:
    scores[l_chunk, m] = qT[:, l_chunk].T @ kT        (fp32, N=256)
followed by one fused DVE pass per chunk:
    out = (scores * mask/sqrt(D)) * V
All tensors stream through SBUF in 8-pair groups (1-2MB DMAs, 1KB contiguous
runs per partition, double-buffered). The kernel is DMA-bound: ~100MB HBM
traffic per core at ~360GB/s.
"""

import numpy as np

B, P, L, D = 16, 64, 256, 128
NCORES = 8
BPC = B // NCORES          # batches per core = 2
PAIRS = BPC * P            # (b,p) pairs per core = 128
GP = 8                     # pairs per DMA group

ISQRT_D = 1.0 / np.sqrt(D)


def build_bass(pairs=PAIRS, gp=GP, sc_bufs=6, io_bufs=2):
    import concourse.bacc as bacc
    import concourse.mybir as mybir
    import concourse.tile as tile
    from concourse.bass import ds, ts

    f32 = mybir.dt.float32
    groups = pairs // gp
    nc = bacc.Bacc("TRN2")

    qt = nc.dram_tensor("qt", [pairs, D * L], f32, kind="ExternalInput")
    kt = nc.dram_tensor("kt", [pairs, D * L], f32, kind="ExternalInput")
    v = nc.dram_tensor("v", [pairs, L * L], f32, kind="ExternalInput")
    # maskbc[part, pair] = mask[pair] / sqrt(D), same value on all partitions
    maskbc = nc.dram_tensor("maskbc", [128, pairs], f32, kind="ExternalInput")
    out = nc.dram_tensor("out", [pairs, L * L], f32, kind="ExternalOutput")

    mult = mybir.AluOpType.mult

    with tile.TileContext(nc) as tc:
        with (
            tc.tile_pool(name="const", bufs=1) as cp,
            tc.tile_pool(name="io", bufs=io_bufs) as io,
            tc.tile_pool(name="pss", bufs=sc_bufs, space="PSUM") as pss,
        ):
            mask_sb = cp.tile([128, pairs], f32, tag="mask")
            nc.sync.dma_start(out=mask_sb[:], in_=maskbc[:, :])

            for g in range(groups):
                sl = slice(g * gp, (g + 1) * gp)
                # partition = d; free = (pair, l); 1KB contiguous runs
                qn = io.tile([128, gp, 256], f32, tag="qn")
                kn = io.tile([128, gp, 256], f32, tag="kn")
                # partition = l % 128; free = (pair, l//128, m); 1KB runs
                vn = io.tile([128, gp, 2, 256], f32, tag="vn")
                osb = io.tile([128, gp, 2, 256], f32, tag="osb")

                nc.sync.dma_start(
                    out=qn[:], in_=qt[sl, :].rearrange("j (p x) -> p j x", p=128)
                )
                nc.sync.dma_start(
                    out=kn[:], in_=kt[sl, :].rearrange("j (p x) -> p j x", p=128)
                )
                nc.sync.dma_start(
                    out=vn[:],
                    in_=v[sl, :].rearrange("j (c p x) -> p j c x", p=128, c=2),
                )

                for j in range(gp):
                    pair = g * gp + j
                    sc = pss.tile([128, 512], f32, tag="sc")
                    for r in range(2):
                        nc.tensor.matmul(
                            sc[:, ds(r * 256, 256)],
                            lhsT=qn[:, j, ts(r, 128)],
                            rhs=kn[:, j, :],
                            start=True,
                            stop=True,
                        )
                        nc.vector.scalar_tensor_tensor(
                            out=osb[:, j, r, :],
                            in0=sc[:, ds(r * 256, 256)],
                            scalar=mask_sb[:, ds(pair, 1)],
                            in1=vn[:, j, r, :],
                            op0=mult,
                            op1=mult,
                        )

                nc.sync.dma_start(
                    out=out[sl, :].rearrange("j (c p x) -> p j c x", p=128, c=2),
                    in_=osb[:],
                )
    nc.finalize()
    return nc


def make_in_maps(queries, keys, values, mask, ncores=NCORES):
    in_maps = []
    for c in range(ncores):
        bs = slice(c * BPC, (c + 1) * BPC)
        mrow = (mask[bs].reshape(PAIRS) * ISQRT_D).astype(np.float32)
        qs = queries[bs].reshape(PAIRS, L, D)
        ks = keys[bs].reshape(PAIRS, L, D)
        in_maps.append(
            {
                "qt": np.ascontiguousarray(qs.transpose(0, 2, 1)).reshape(
                    PAIRS, D * L
                ),
                "kt": np.ascontiguousarray(ks.transpose(0, 2, 1)).reshape(
                    PAIRS, D * L
                ),
                "v": np.ascontiguousarray(
                    values[bs].reshape(PAIRS, L * L).astype(np.float32)
                ),
                "maskbc": np.ascontiguousarray(
                    np.broadcast_to(mrow[None, :], (128, PAIRS))
                ),
            }
        )
    return in_maps


def run(queries, keys, values, mask, trace=False, **build_kwargs):
    """Build, compile and run on 8 cores; returns (full_output, BassKernelResults)."""
    from concourse.bass_utils import run_bass_kernel_spmd

    nc = build_bass(**build_kwargs)
    in_maps = make_in_maps(queries, keys, values, mask)
    res = run_bass_kernel_spmd(
        nc, in_maps, core_ids=list(range(NCORES)), trace=trace
    )
    outs = [r["out"].reshape(BPC, P, L, L) for r in res.results]
    return np.concatenate(outs, axis=0), res


def kernel(queries, keys, values, mask):
    out, _ = run(queries, keys, values, mask, trace=False)
    return out


# revision 12
# speedup vs baseline: 1.1035x; 1.0643x over previous
"""Trainium2 Bass kernel for masked dot-product attention variant:

    out[b,p,l,m] = (sum_d Q[b,p,l,d] K[b,p,m,d]) / sqrt(D) * mask[b,p] * V[b,p,l,m]

Sharding: data-parallel over batch dim B=16 -> 2 batches per core on 8 cores.
Per core: 128 independent (b,p) pairs, each a 256x128 @ 128x256 fp32 gemm plus
an elementwise multiply with V and a per-pair scalar mask.

Host marshalling pre-transposes Q,K to [pair, d, l] layout so the PE matmul
(which contracts along the partition dim) can consume both operands directly:
    scores[l_chunk, m] = qT[:, l_chunk].T @ kT        (fp32, N=256)
followed by one fused DVE pass per chunk:
    out = (scores * mask/sqrt(D)) * V
All tensors stream through SBUF in 8-pair groups (1-2MB DMAs, 1KB contiguous
runs per partition, double-buffered). The kernel is DMA-bound: ~100MB HBM
traffic per core at ~360GB/s.
"""

import numpy as np

B, P, L, D = 16, 64, 256, 128
NCORES = 8
BPC = B // NCORES          # batches per core = 2
PAIRS = BPC * P            # (b,p) pairs per core = 128
GP = 8                     # pairs per DMA group

ISQRT_D = 1.0 / np.sqrt(D)


def build_bass(pairs=PAIRS, gp=GP, sc_bufs=6, io_bufs=3):
    import concourse.bacc as bacc
    import concourse.mybir as mybir
    import concourse.tile as tile
    from concourse.bass import ds, ts

    f32 = mybir.dt.float32
    groups = pairs // gp
    nc = bacc.Bacc("TRN2")

    qt = nc.dram_tensor("qt", [pairs, D * L], f32, kind="ExternalInput")
    kt = nc.dram_tensor("kt", [pairs, D * L], f32, kind="ExternalInput")
    v = nc.dram_tensor("v", [pairs, L * L], f32, kind="ExternalInput")
    # maskbc[part, pair] = mask[pair] / sqrt(D), same value on all partitions
    maskbc = nc.dram_tensor("maskbc", [128, pairs], f32, kind="ExternalInput")
    out = nc.dram_tensor("out", [pairs, L * L], f32, kind="ExternalOutput")

    mult = mybir.AluOpType.mult

    with tile.TileContext(nc) as tc:
        with (
            tc.tile_pool(name="const", bufs=1) as cp,
            tc.tile_pool(name="io", bufs=io_bufs) as io,
            tc.tile_pool(name="pss", bufs=sc_bufs, space="PSUM") as pss,
        ):
            mask_sb = cp.tile([128, pairs], f32, tag="mask")
            nc.sync.dma_start(out=mask_sb[:], in_=maskbc[:, :])

            for g in range(groups):
                sl = slice(g * gp, (g + 1) * gp)
                # partition = d; free = (pair, l); 1KB contiguous runs.
                # qt columns are host-interleaved: col (r,p) holds l = 2p+r
                qn = io.tile([128, gp, 256], f32, tag="qn")
                kn = io.tile([128, gp, 256], f32, tag="kn")
                # v/out flat: partition p holds rows l = 2p, 2p+1 (2KB runs)
                vn = io.tile([128, gp, 2, 256], f32, tag="vn")
                osb = io.tile([128, gp, 2, 256], f32, tag="osb")

                nc.sync.dma_start(
                    out=qn[:], in_=qt[sl, :].rearrange("j (p x) -> p j x", p=128)
                )
                nc.sync.dma_start(
                    out=kn[:], in_=kt[sl, :].rearrange("j (p x) -> p j x", p=128)
                )
                nc.sync.dma_start(
                    out=vn[:],
                    in_=v[sl, :].rearrange("j (p c x) -> p j c x", p=128, c=2),
                )

                for j in range(gp):
                    pair = g * gp + j
                    sc = pss.tile([128, 512], f32, tag="sc")
                    for r in range(2):
                        nc.tensor.matmul(
                            sc[:, ds(r * 256, 256)],
                            lhsT=qn[:, j, ts(r, 128)],
                            rhs=kn[:, j, :],
                            start=True,
                            stop=True,
                        )
                        nc.vector.scalar_tensor_tensor(
                            out=osb[:, j, r, :],
                            in0=sc[:, ds(r * 256, 256)],
                            scalar=mask_sb[:, ds(pair, 1)],
                            in1=vn[:, j, r, :],
                            op0=mult,
                            op1=mult,
                        )

                nc.sync.dma_start(
                    out=out[sl, :].rearrange("j (p c x) -> p j c x", p=128, c=2),
                    in_=osb[:],
                )
    nc.finalize()
    return nc


def make_in_maps(queries, keys, values, mask, ncores=NCORES):
    in_maps = []
    for c in range(ncores):
        bs = slice(c * BPC, (c + 1) * BPC)
        mrow = (mask[bs].reshape(PAIRS) * ISQRT_D).astype(np.float32)
        qs = queries[bs].reshape(PAIRS, L, D)
        ks = keys[bs].reshape(PAIRS, L, D)
        # qt columns interleaved so score chunk r's partition p is row l=2p+r,
        # matching the flat (2KB-run) V/out layout. [pair, d, r, p] = QT[d, 2p+r]
        qt = qs.transpose(0, 2, 1).reshape(PAIRS, D, 128, 2).transpose(0, 1, 3, 2)
        in_maps.append(
            {
                "qt": np.ascontiguousarray(qt).reshape(PAIRS, D * L),
                "kt": np.ascontiguousarray(ks.transpose(0, 2, 1)).reshape(
                    PAIRS, D * L
                ),
                "v": np.ascontiguousarray(
                    values[bs].reshape(PAIRS, L * L).astype(np.float32)
                ),
                "maskbc": np.ascontiguousarray(
                    np.broadcast_to(mrow[None, :], (128, PAIRS))
                ),
            }
        )
    return in_maps


def run(queries, keys, values, mask, trace=False, **build_kwargs):
    """Build, compile and run on 8 cores; returns (full_output, BassKernelResults)."""
    from concourse.bass_utils import run_bass_kernel_spmd

    nc = build_bass(**build_kwargs)
    in_maps = make_in_maps(queries, keys, values, mask)
    res = run_bass_kernel_spmd(
        nc, in_maps, core_ids=list(range(NCORES)), trace=trace
    )
    outs = [r["out"].reshape(BPC, P, L, L) for r in res.results]
    return np.concatenate(outs, axis=0), res


def kernel(queries, keys, values, mask):
    out, _ = run(queries, keys, values, mask, trace=False)
    return out
